# revision 4
# baseline (speedup 1.0000x reference)
"""Trainium2 Bass kernel for multi-head self-attention with Q=K=V=x@Wq.

Problem: x [4, 2048, 512] f32, Wq [512, 512] f32, HEAD=8 (head_dim=64).
  q = x @ Wq;  per (b, h): S = q_h q_h^T / 8; out = softmax(S) @ q_h.

Sharding (8 cores): core i -> batch b = i//2, head group g = i%2 (4 heads).
Each core gets x[b] [2048,512] and Wq[:, 256g:256g+256] [512,256]; produces
out[b, :, 256g:256g+256].  No cross-core communication.

On-core algorithm (v2):
  Startup (fused per 128-row block ib):
    - DMA x block, 4 PE transposes -> xtp PSUM, ScalarE copy -> xt_sb (f32r)
    - q_nat(ib) = x(ib) @ Wq via 4 PE matmuls (contract d), GpSimd copy into
      q_nat [128, 16, 4, 65] f32r whose 65th column is pre-set to 1.0
    - q_T(ib) per head via 4 PE transposes of q_nat(ib), DVE copy -> q_T
  Main loop per (h, half=1024 queries), 16 key blocks jb:
    - S[a,b] = q_T_jb^T q_T_half (2 f32r matmuls N=512) -> sp PSUM
    - eb = exp(0.125*S) via ScalarE ACT (PSUM->SBUF, f32r), no accumulator
    - ctx_T[d|Z, i] += q_nat[jb, h, 0:65]^T eb: the 65th (ones) stationary
      column makes PSUM row 64 accumulate Z_i = sum_a E[a, i] for free
    - emitted S(jb)/exp(jb) before ctx(jb-1) so PE overlaps ACT
  Tail per (h, half), interleaved into the NEXT (h, half)'s jb loop:
    - DVE copy cp [65,1024] -> SBUF; 8 PE transposes [65,128] -> [128,65];
      DVE reciprocal of col 64 -> rz; DVE tensor_scalar_mul; DMA out.
  No max-subtraction needed: diag(S)/8 ~ 8 dominates, exp stays in f32 range.
"""

import sys

sys.path.insert(0, "/opt/trn_rl_repo")

from contextlib import ExitStack

import numpy as np

import concourse.bass as bass
import concourse.tile as tile
from concourse import bacc, mybir
from concourse.masks import make_identity

B, S, D, HEAD = 4, 2048, 512, 8
HD = D // HEAD  # 64
EC = 256  # e-columns per core (4 heads)
F32 = mybir.dt.float32
# float32r: single-pass PE matmul (~2x faster than fp32) for all main-loop
# matmul operands.  f32r operands must sit at SBUF partition 0 (ISA check).
F32R = mybir.dt.float32r
N_CORES = 8

_PROGRAM = None


def build_program():
    nc = bacc.Bacc(None, target_bir_lowering=False)
    x_d = nc.dram_tensor("x", [S, D], F32, kind="ExternalInput")
    wq_d = nc.dram_tensor("wq", [D, EC], F32, kind="ExternalInput")
    out_d = nc.dram_tensor("out", [S, EC], F32, kind="ExternalOutput")

    x_r = x_d.rearrange("(ib p) d -> ib p d", p=128)  # [16, 128, 512]
    wq_r = wq_d.rearrange("(dc p) e -> p dc e", p=128)  # [128, 4, 256]
    out_r = out_d.rearrange("(ib p) e -> ib p e", p=128)  # [16, 128, 256]

    with tile.TileContext(nc) as tc, ExitStack() as ctx:
        sb = ctx.enter_context(tc.tile_pool(name="sb", bufs=1))
        xb = ctx.enter_context(tc.tile_pool(name="xb", bufs=3))
        xsb = ctx.enter_context(tc.tile_pool(name="xsb", bufs=3))
        ep = ctx.enter_context(tc.tile_pool(name="ep", bufs=6))
        csbp = ctx.enter_context(tc.tile_pool(name="csbp", bufs=2))
        ob = ctx.enter_context(tc.tile_pool(name="ob", bufs=6))
        rzp = ctx.enter_context(tc.tile_pool(name="rzp", bufs=6))
        ps = ctx.enter_context(tc.tile_pool(name="ps", bufs=3, space="PSUM"))
        cps = ctx.enter_context(tc.tile_pool(name="cps", bufs=1, space="PSUM"))

        ident = sb.tile([128, 128], F32)
        make_identity(nc, ident)
        ident_r = sb.tile([128, 128], F32R)
        nc.vector.tensor_copy(ident_r, ident)

        wq_f32 = sb.tile([128, 4, EC], F32)
        nc.sync.dma_start(out=wq_f32, in_=wq_r)
        wq_sb = sb.tile([128, 4, EC], F32R)
        nc.vector.tensor_copy(wq_sb, wq_f32)

        # HAM ignition: the PE powers on clock-throttled to 1.2 GHz and only
        # reaches 2.4 GHz after ~3.4us of a fully wait-free matmul stream.
        # Burn ~10us of dummy back-to-back matmuls up front (overlapped with
        # the x DMA) to ignite; the busy main loop never re-throttles.
        wrm = sb.tile([128, 512], F32)
        nc.vector.memset(wrm, 0.0)
        wps = ps.tile([128, 512], F32, tag="ps")
        for i in range(14):
            nc.tensor.matmul(wps, ident, wrm, start=True, stop=True)
        # Trigger the one-time ~2.7us exp table load NOW (ACT is idle).
        dume = sb.tile([128, 1], F32)
        nc.scalar.activation(dume, wrm[:, 0:1], mybir.ActivationFunctionType.Exp)

        # Persistent SBUF: q_T per head (e on partitions 0..63, f32r base 0),
        # q_nat with a 65th all-ones column per (jb, h) for the free Z row.
        q_T = sb.tile([64, 4, S], F32R)  # [e_in_head, h, i]
        q_nat = sb.tile([128, 16, 4, 65], F32R)  # [j_in_block, jb, h, e|1]
        nc.vector.memset(q_nat[:, :, :, 64:65].bitcast(F32), 1.0)

        # ---- Startup: fused per-block x load, x^T, q_nat, q_T ----
        for ib in range(16):
            xt = xb.tile([128, D], F32, tag="xt")
            nc.sync.dma_start(out=xt, in_=x_r[ib])
            xtp = ps.tile([128, 512], F32, tag="ps")
            for dc in range(4):
                nc.tensor.transpose(
                    xtp[:, dc * 128 : (dc + 1) * 128],
                    xt[:, dc * 128 : (dc + 1) * 128],
                    ident,
                )
            xt_sb = xsb.tile([128, 4, 128], F32R, tag="xs")  # [d_chunk, dc, i]
            nc.scalar.copy(xt_sb, xtp)

            qn = ps.tile([128, 4, 64], F32, tag="ps")
            for dc in range(4):
                nc.tensor.matmul(
                    qn,
                    xt_sb[:, dc, :],
                    wq_sb[:, dc, :],
                    start=(dc == 0),
                    stop=(dc == 3),
                )
            nc.vector.tensor_copy(q_nat[:, ib, :, 0:64], qn)

            qtp = ps.tile([64, 4, 128], F32R, tag="ps")
            for h in range(4):
                nc.tensor.transpose(
                    qtp[:, h, :], q_nat[:, ib, h, 0:64], ident_r
                )
            nc.vector.tensor_copy(
                q_T[:, :, ib * 128 : (ib + 1) * 128], qtp
            )

        # ---- Main loop + interleaved tails ----
        pending_tail = []  # list of closures, 2 drained per jb iteration

        def make_tail(h, half, cp):
            csb = csbp.tile([65, 1024], F32, tag="csb")

            def piece_copy():
                nc.vector.tensor_copy(csb, cp)

            def make_piece(icc):
                def piece():
                    tp = ps.tile([128, 65], F32, tag="ps")
                    nc.tensor.transpose(
                        tp, csb[:, icc * 128 : (icc + 1) * 128], ident[0:65, 0:65]
                    )
                    rz = rzp.tile([128, 1], F32, tag="rz")
                    nc.vector.reciprocal(rz, tp[:, 64:65])
                    ot = ob.tile([128, 64], F32, tag="ot")
                    nc.vector.tensor_scalar_mul(ot, tp[:, 0:64], rz)
                    nc.sync.dma_start(
                        out=out_r[half * 8 + icc, :, h * 64 : (h + 1) * 64], in_=ot
                    )

                return piece

            return [piece_copy] + [make_piece(i) for i in range(8)]

        for h in range(4):
            for half in range(2):
                cp = cps.tile([65, 1024], F32, tag="ctx")

                def ctx_mms(jb, eb):
                    for nn in range(2):
                        nc.tensor.matmul(
                            cp[:, nn * 512 : (nn + 1) * 512],
                            q_nat[:, jb, h, 0:65],
                            eb[:, nn * 512 : (nn + 1) * 512],
                            start=(jb == 0),
                            stop=(jb == 15),
                        )

                pending = None
                for jb in range(16):  # key block (rows a)
                    sp = ps.tile([128, 1024], F32, tag="ps")
                    for nn in range(2):
                        nc.tensor.matmul(
                            sp[:, nn * 512 : (nn + 1) * 512],
                            q_T[:, h, jb * 128 : (jb + 1) * 128],
                            q_T[
                                :,
                                h,
                                half * 1024 + nn * 512 : half * 1024 + (nn + 1) * 512,
                            ],
                            start=True,
                            stop=True,
                        )
                    eb = ep.tile([128, 1024], F32R, tag="eb")
                    nc.scalar.activation(
                        eb, sp, mybir.ActivationFunctionType.Exp, scale=0.125
                    )
                    if pending is not None:
                        ctx_mms(*pending)
                    # drain up to 2 tail pieces of the previous (h, half)
                    for _ in range(2):
                        if pending_tail:
                            pending_tail.pop(0)()
                    pending = (jb, eb)
                ctx_mms(*pending)

                assert not pending_tail
                pending_tail = make_tail(h, half, cp)

        # last (h, half): drain remaining tail pieces
        for piece in pending_tail:
            piece()

    nc.compile()
    return nc


def get_program():
    global _PROGRAM
    if _PROGRAM is None:
        _PROGRAM = build_program()
    return _PROGRAM


def make_in_maps(x, Wq):
    x = np.asarray(x, dtype=np.float32)
    Wq = np.asarray(Wq, dtype=np.float32)
    in_maps = []
    for core in range(N_CORES):
        b, g = core // 2, core % 2
        in_maps.append(
            {
                "x": np.ascontiguousarray(x[b]),
                "wq": np.ascontiguousarray(Wq[:, g * EC : (g + 1) * EC]),
            }
        )
    return in_maps


def assemble(results):
    out = np.empty((B, S, D), dtype=np.float32)
    for core in range(N_CORES):
        b, g = core // 2, core % 2
        out[b, :, g * EC : (g + 1) * EC] = results[core]["out"]
    return out


def kernel(x, Wq):
    from concourse.bass_utils import run_bass_kernel_spmd

    nc = get_program()
    res = run_bass_kernel_spmd(nc, make_in_maps(x, Wq), list(range(N_CORES)))
    return assemble(res.results)


# revision 7
# speedup vs baseline: 1.1205x; 1.1205x over previous
"""Trainium2 Bass kernel for multi-head self-attention with Q=K=V=x@Wq.

Problem: x [4, 2048, 512] f32, Wq [512, 512] f32, HEAD=8 (head_dim=64).
  q = x @ Wq;  per (b, h): S = q_h q_h^T / 8; out = softmax(S) @ q_h.

Sharding (8 cores): core i -> batch b = i//2, head group g = i%2 (4 heads).
Each core gets x[b] [2048,512] and Wq[:, 256g:256g+256] [512,256]; produces
out[b, :, 256g:256g+256].  No cross-core communication.

On-core algorithm (v2):
  Startup (fused per 128-row block ib):
    - DMA x block, 4 PE transposes -> xtp PSUM, ScalarE copy -> xt_sb (f32r)
    - q_nat(ib) = x(ib) @ Wq via 4 PE matmuls (contract d), GpSimd copy into
      q_nat [128, 16, 4, 65] f32r whose 65th column is pre-set to 1.0
    - q_T(ib) per head via 4 PE transposes of q_nat(ib), DVE copy -> q_T
  Main loop per (h, half=1024 queries), 16 key blocks jb:
    - S[a,b] = q_T_jb^T q_T_half (2 f32r matmuls N=512) -> sp PSUM
    - eb = exp(0.125*S) via ScalarE ACT (PSUM->SBUF, f32r), no accumulator
    - ctx_T[d|Z, i] += q_nat[jb, h, 0:65]^T eb: the 65th (ones) stationary
      column makes PSUM row 64 accumulate Z_i = sum_a E[a, i] for free
    - emitted S(jb)/exp(jb) before ctx(jb-1) so PE overlaps ACT
  Tail per (h, half), interleaved into the NEXT (h, half)'s jb loop:
    - DVE copy cp [65,1024] -> SBUF; 8 PE transposes [65,128] -> [128,65];
      DVE reciprocal of col 64 -> rz; DVE tensor_scalar_mul; DMA out.
  No max-subtraction needed: diag(S)/8 ~ 8 dominates, exp stays in f32 range.
"""

import sys

sys.path.insert(0, "/opt/trn_rl_repo")

from contextlib import ExitStack

import numpy as np

import concourse.bass as bass
import concourse.tile as tile
from concourse import bacc, mybir
from concourse.masks import make_identity

B, S, D, HEAD = 4, 2048, 512, 8
HD = D // HEAD  # 64
EC = 256  # e-columns per core (4 heads)
F32 = mybir.dt.float32
# float32r: single-pass PE matmul (~2x faster than fp32) for all main-loop
# matmul operands.  f32r operands must sit at SBUF partition 0 (ISA check).
F32R = mybir.dt.float32r
N_CORES = 8

_PROGRAM = None


def build_program():
    nc = bacc.Bacc(None, target_bir_lowering=False)
    x_d = nc.dram_tensor("x", [S, D], F32, kind="ExternalInput")
    wq_d = nc.dram_tensor("wq", [D, EC], F32, kind="ExternalInput")
    out_d = nc.dram_tensor("out", [S, EC], F32, kind="ExternalOutput")

    x_r = x_d.rearrange("(ib p) d -> ib p d", p=128)  # [16, 128, 512]
    wq_r = wq_d.rearrange("(dc p) e -> p dc e", p=128)  # [128, 4, 256]
    out_r = out_d.rearrange("(ib p) e -> ib p e", p=128)  # [16, 128, 256]

    with tile.TileContext(nc) as tc, ExitStack() as ctx:
        sb = ctx.enter_context(tc.tile_pool(name="sb", bufs=1))
        xb = ctx.enter_context(tc.tile_pool(name="xb", bufs=3))
        xsb = ctx.enter_context(tc.tile_pool(name="xsb", bufs=3))
        ep = ctx.enter_context(tc.tile_pool(name="ep", bufs=6))
        csbp = ctx.enter_context(tc.tile_pool(name="csbp", bufs=2))
        ob = ctx.enter_context(tc.tile_pool(name="ob", bufs=6))
        rzp = ctx.enter_context(tc.tile_pool(name="rzp", bufs=6))
        ps = ctx.enter_context(tc.tile_pool(name="ps", bufs=3, space="PSUM"))
        cps = ctx.enter_context(tc.tile_pool(name="cps", bufs=1, space="PSUM"))

        ident = sb.tile([128, 128], F32)
        make_identity(nc, ident)
        ident_r = sb.tile([128, 128], F32R)
        nc.vector.tensor_copy(ident_r, ident)

        wq_f32 = sb.tile([128, 4, EC], F32)
        nc.sync.dma_start(out=wq_f32, in_=wq_r)
        wq_sb = sb.tile([128, 4, EC], F32R)
        nc.vector.tensor_copy(wq_sb, wq_f32)

        # HAM ignition: the PE powers on clock-throttled to 1.2 GHz and only
        # reaches 2.4 GHz after ~3.4us of a fully wait-free matmul stream.
        # Burn ~10us of dummy back-to-back matmuls up front (overlapped with
        # the x DMA) to ignite; the busy main loop never re-throttles.
        wrm = sb.tile([128, 512], F32)
        nc.vector.memset(wrm, 0.0)
        wps = ps.tile([128, 512], F32, tag="ps")
        for i in range(14):
            nc.tensor.matmul(wps, ident, wrm, start=True, stop=True)
        # Trigger the one-time ~2.7us exp table load NOW (ACT is idle).
        dume = sb.tile([128, 1], F32)
        nc.scalar.activation(dume, wrm[:, 0:1], mybir.ActivationFunctionType.Exp)

        # Persistent SBUF: q_T per head (e on partitions 0..63, f32r base 0),
        # q_nat with a 65th all-ones column per (jb, h) for the free Z row.
        q_T = sb.tile([64, 4, S], F32R)  # [e_in_head, h, i]
        q_nat = sb.tile([128, 16, 4, 65], F32R)  # [j_in_block, jb, h, e|1]
        nc.vector.memset(q_nat[:, :, :, 64:65].bitcast(F32), 1.0)

        # ---- Startup: software-pipelined x load, x^T, q_nat, q_T ----
        # Stages for block i run at steps i, i+1, i+2 so the PE stream stays
        # dense (any >~3.4us of PE waits trips the 50%-utilization throttle,
        # which then never lifts during the sem-synced main loop).
        xt_sbs = {}
        for step in range(18):
            if step < 16:
                ib = step
                xt = xb.tile([128, D], F32, tag="xt")
                nc.sync.dma_start(out=xt, in_=x_r[ib])
                xtp = ps.tile([128, 512], F32, tag="ps")
                for dc in range(4):
                    nc.tensor.transpose(
                        xtp[:, dc * 128 : (dc + 1) * 128],
                        xt[:, dc * 128 : (dc + 1) * 128],
                        ident,
                    )
                xt_sb = xsb.tile([128, 4, 128], F32R, tag="xs")  # [d, dc, i]
                nc.scalar.copy(xt_sb, xtp)
                xt_sbs[ib] = xt_sb
            if 1 <= step <= 16:
                jb = step - 1
                qn = ps.tile([128, 4, 64], F32, tag="ps")
                for dc in range(4):
                    nc.tensor.matmul(
                        qn,
                        xt_sbs[jb][:, dc, :],
                        wq_sb[:, dc, :],
                        start=(dc == 0),
                        stop=(dc == 3),
                    )
                del xt_sbs[jb]
                nc.vector.tensor_copy(q_nat[:, jb, :, 0:64], qn)
            if 2 <= step <= 17:
                kb = step - 2
                qtp = ps.tile([64, 4, 128], F32R, tag="ps")
                for h in range(4):
                    nc.tensor.transpose(
                        qtp[:, h, :], q_nat[:, kb, h, 0:64], ident_r
                    )
                nc.vector.tensor_copy(
                    q_T[:, :, kb * 128 : (kb + 1) * 128], qtp
                )

        # Re-ignition burst: ~4us of wait-free back-to-back matmuls to make
        # sure the PE is at full clock entering the main loop.
        wps2 = ps.tile([128, 512], F32, tag="ps")
        for i in range(10):
            nc.tensor.matmul(wps2, ident, wrm, start=True, stop=True)

        # ---- Main loop + interleaved tails ----
        pending_tail = []  # list of closures, 2 drained per jb iteration

        def make_tail(h, half, cp):
            csb = csbp.tile([65, 1024], F32, tag="csb")

            def piece_copy():
                nc.vector.tensor_copy(csb, cp)

            def make_piece(icc):
                def piece():
                    tp = ps.tile([128, 65], F32, tag="ps")
                    nc.tensor.transpose(
                        tp, csb[:, icc * 128 : (icc + 1) * 128], ident[0:65, 0:65]
                    )
                    rz = rzp.tile([128, 1], F32, tag="rz")
                    nc.vector.reciprocal(rz, tp[:, 64:65])
                    ot = ob.tile([128, 64], F32, tag="ot")
                    nc.vector.tensor_scalar_mul(ot, tp[:, 0:64], rz)
                    nc.sync.dma_start(
                        out=out_r[half * 8 + icc, :, h * 64 : (h + 1) * 64], in_=ot
                    )

                return piece

            return [piece_copy] + [make_piece(i) for i in range(8)]

        for h in range(4):
            for half in range(2):
                cp = cps.tile([65, 1024], F32, tag="ctx")

                def ctx_mms(jb, eb):
                    for nn in range(2):
                        nc.tensor.matmul(
                            cp[:, nn * 512 : (nn + 1) * 512],
                            q_nat[:, jb, h, 0:65],
                            eb[:, nn * 512 : (nn + 1) * 512],
                            start=(jb == 0),
                            stop=(jb == 15),
                        )

                pending = None
                for jb in range(16):  # key block (rows a)
                    sp = ps.tile([128, 1024], F32, tag="ps")
                    for nn in range(2):
                        nc.tensor.matmul(
                            sp[:, nn * 512 : (nn + 1) * 512],
                            q_T[:, h, jb * 128 : (jb + 1) * 128],
                            q_T[
                                :,
                                h,
                                half * 1024 + nn * 512 : half * 1024 + (nn + 1) * 512,
                            ],
                            start=True,
                            stop=True,
                        )
                    eb = ep.tile([128, 1024], F32R, tag="eb")
                    nc.scalar.activation(
                        eb, sp, mybir.ActivationFunctionType.Exp, scale=0.125
                    )
                    if pending is not None:
                        ctx_mms(*pending)
                    # drain up to 2 tail pieces of the previous (h, half)
                    for _ in range(2):
                        if pending_tail:
                            pending_tail.pop(0)()
                    pending = (jb, eb)
                ctx_mms(*pending)

                assert not pending_tail
                pending_tail = make_tail(h, half, cp)

        # last (h, half): drain remaining tail pieces
        for piece in pending_tail:
            piece()

    nc.compile()
    return nc


def get_program():
    global _PROGRAM
    if _PROGRAM is None:
        _PROGRAM = build_program()
    return _PROGRAM


def make_in_maps(x, Wq):
    x = np.asarray(x, dtype=np.float32)
    Wq = np.asarray(Wq, dtype=np.float32)
    in_maps = []
    for core in range(N_CORES):
        b, g = core // 2, core % 2
        in_maps.append(
            {
                "x": np.ascontiguousarray(x[b]),
                "wq": np.ascontiguousarray(Wq[:, g * EC : (g + 1) * EC]),
            }
        )
    return in_maps


def assemble(results):
    out = np.empty((B, S, D), dtype=np.float32)
    for core in range(N_CORES):
        b, g = core // 2, core % 2
        out[b, :, g * EC : (g + 1) * EC] = results[core]["out"]
    return out


def kernel(x, Wq):
    from concourse.bass_utils import run_bass_kernel_spmd

    nc = get_program()
    res = run_bass_kernel_spmd(nc, make_in_maps(x, Wq), list(range(N_CORES)))
    return assemble(res.results)
